# revision 38
# baseline (speedup 1.0000x reference)
"""Trainium2 Bass kernel for nn_CPF_prop_f_87144886436370 (moe_routing).

Per row r of x[N=262144, C=128]:
  xn = (x_r - mean_r) / sqrt(var_r(ddof=1) + 1)
  y  = xn @ W[:, :, labels_r]          (W: [C, C, P=8])
  out_r = y - tanh(y)                   (tanhshrink)

Strategy: data-parallel over 8 NeuronCores (32768 rows each). On each core,
per 128-row tile: layernorm stats + Newton rsqrt + normalize on DVE, PE
transpose, fp32 matmul against all 8 cluster matrices stacked [128, 1024],
per-row selection of the labeled 128-column block via copy_predicated,
tanhshrink (ACT tanh + DVE subtract), store.

Toolchain note: this walrus build allows very few semaphore waits per
instruction, so the kernel is structured to keep every instruction at a
single wait: the x shard is preloaded into SBUF with fresh-region DMAs, PE
warm-up ops absorb one-time cross-engine deps, the ACT engine only ever runs
Tanh (no table switches) and writes into the per-tile dead x_sb column (no
slot rotation → no WAW self-waits), and rsqrt is computed on DVE by Newton
iteration instead of ACT Sqrt.
"""

import numpy as np

import concourse.bass as bass
import concourse.tile as tile
from concourse import bacc, mybir
from concourse.bass import ts
from concourse.bass_utils import run_bass_kernel_spmd
from concourse.masks import make_identity

N = 262144
C = 128
P = 8
N_CORES = 8
ROWS_PER_CORE = N // N_CORES          # 32768
TILES = ROWS_PER_CORE // 128          # 256
FB = 8                                # stats blocking factor
VAR_SCALE = C / (C - 1.0)             # unbiased correction on biased bn var
EPS = 1.0
MAGIC = 0x5F3759DF

F32 = mybir.dt.float32
I32 = mybir.dt.int32
OP = mybir.AluOpType

_NC_CACHE = {}


def _build_kernel():
    # Bacc (not plain Bass): its compile() pass splits semaphore waits to
    # one per instruction, which this walrus build requires.
    nc = bacc.Bacc(target_bir_lowering=False, debug=False)
    x = nc.declare_dram_parameter("x", [ROWS_PER_CORE, C], F32, isOutput=False)
    labels_t = nc.declare_dram_parameter("labels_t", [128, TILES], F32, isOutput=False)
    w_cat = nc.declare_dram_parameter("w_cat", [C, P * C], F32, isOutput=False)
    out = nc.declare_dram_parameter("out", [ROWS_PER_CORE, C], F32, isOutput=True)

    with tile.TileContext(nc) as tc:
        with (
            tc.tile_pool(name="singles", bufs=1) as singles,
            tc.tile_pool(name="temps", bufs=3) as temps,
            tc.tile_pool(name="stats", bufs=3) as statsp,
            tc.tile_pool(name="psum_t", bufs=2, space="PSUM") as psum_t_pool,
            tc.tile_pool(name="psum_mm", bufs=2, space="PSUM") as psum_mm_pool,
            tc.tile_pool(name="psum_w", bufs=1, space="PSUM") as psum_w_pool,
        ):
            # One-time setup
            w_sb = singles.tile([C, P * C], F32)
            nc.sync.dma_start(out=w_sb, in_=w_cat[:, :])
            labels_sb = singles.tile([128, TILES], F32)
            nc.sync.dma_start(out=labels_sb, in_=labels_t[:, :])
            ident = singles.tile([128, 128], F32)
            make_identity(nc, ident[:])
            zero_t = singles.tile([128, 1], F32)
            nc.vector.memset(zero_t[:], 0.0)
            # Per-cluster one-hot masks: mask8[r, c, t] (int mask for
            # CopyPredicated)
            mask8 = singles.tile([128, P, TILES], mybir.dt.uint8)
            for c in range(P):
                nc.vector.tensor_scalar(
                    mask8[:, c, :], labels_sb[:, :], float(c), None,
                    OP.is_equal,
                )

            # Preload the whole x shard into SBUF (64KB/partition) with
            # fresh-region DMAs — no load-slot reuse (DMACopy has a single
            # wait slot and slot-reuse WAW waits would overflow it).
            x_sb = singles.tile([128, TILES, C], F32)
            x_view = x[:, :].rearrange("(t r) c -> r t c", r=128)
            NCH = 16
            chw = TILES // NCH
            for ch in range(NCH):
                nc.sync.dma_start(
                    out=x_sb[:, ch * chw:(ch + 1) * chw, :],
                    in_=x_view[:, ch * chw:(ch + 1) * chw, :])
            # tanh results also go into dead x_sb columns (fresh bytes per
            # tile → no rotating-slot WAW on the ACT engine). Warm the ACT
            # engine on each preload DMA lane so the per-tile tanh carries
            # only its DVE data wait.
            act_warm = singles.tile([128, NCH], F32)
            for ch in range(NCH):
                nc.scalar.copy(out=act_warm[:, ch:ch + 1],
                               in_=x_sb[:, ch * chw, 0:1])
            # tanh for the 16 warm-probed tiles goes to fresh scratch instead
            # (the warm read would otherwise add an ACT WAR wait there)
            th_scratch = singles.tile([128, NCH, 128], F32)

            # PE warm-ups: absorb one-time cross-engine deps (identity from
            # GPSIMD, weights from DMA).
            ps_warm_t = psum_w_pool.tile([128, 128], F32, tag="warm_t")
            nc.tensor.transpose(ps_warm_t[:], ident[:], ident[:])
            ps_warm_m = psum_w_pool.tile([128, 512], F32, tag="warm_m")
            nc.tensor.matmul(ps_warm_m[:], lhsT=w_sb[:, 0:128],
                             rhs=w_sb[:, 0:512], start=True, stop=True)

            n_blocks = TILES // FB
            for blk in range(n_blocks):
                # --- blocked stats: bn stats per tile, rsqrt per block ---
                mv_blk = statsp.tile([128, FB, 2], F32, tag="mv")
                for f in range(FB):
                    t = blk * FB + f
                    stats = statsp.tile([128, 6], F32, tag="bst")
                    nc.vector.bn_stats(out=stats, in_=x_sb[:, t, :])
                    nc.vector.bn_aggr(out=mv_blk[:, f, :], in_=stats)
                # vp = var * C/(C-1) + EPS   [128, FB]
                vp = statsp.tile([128, FB], F32, tag="vp")
                nc.vector.tensor_scalar(
                    vp, mv_blk[:, :, 1], VAR_SCALE, EPS, OP.mult, OP.add)
                # rstd = rsqrt(vp) by magic seed + 3 Newton steps (DVE only)
                vpi = vp[:, :].bitcast(I32)
                yi = statsp.tile([128, FB], I32, tag="yi")
                nc.vector.tensor_scalar(yi, vpi, 1, None, OP.arith_shift_right)
                nc.vector.tensor_scalar(yi, yi, -1, MAGIC, OP.mult, OP.add)
                y = yi[:, :].bitcast(F32)
                tmp = statsp.tile([128, FB], F32, tag="tmp")
                for _ in range(3):
                    nc.vector.tensor_tensor(out=tmp, in0=y, in1=y, op=OP.mult)
                    nc.vector.tensor_tensor(out=tmp, in0=tmp, in1=vp, op=OP.mult)
                    nc.vector.tensor_scalar(tmp, tmp, -0.5, 1.5, OP.mult, OP.add)
                    nc.vector.tensor_tensor(out=y, in0=y, in1=tmp, op=OP.mult)

                for f in range(FB):
                    t = blk * FB + f
                    x_t = x_sb[:, t, :]
                    rstd = y[:, f:f + 1]
                    mean = mv_blk[:, f, 0:1]

                    # xn = (x - mean) * rstd
                    xn = temps.tile([128, C], F32, tag="xn")
                    nc.vector.tensor_scalar(
                        xn, x_t, mean, rstd, OP.subtract, OP.mult)

                    # Transpose xn -> [C, rows]
                    ps_t = psum_t_pool.tile([128, 128], F32, tag="ps_t")
                    nc.tensor.transpose(ps_t[:], xn[:], ident[:])
                    xnT = temps.tile([128, 128], F32, tag="xnT")
                    nc.scalar.copy(out=xnT, in_=ps_t[:])

                    # Candidates for all 8 clusters: [rows, 8*128]
                    ps_a = psum_mm_pool.tile([128, 512], F32, tag="ps_a")
                    ps_b = psum_mm_pool.tile([128, 512], F32, tag="ps_b")
                    nc.tensor.matmul(ps_a[:], lhsT=xnT[:], rhs=w_sb[:, 0:512],
                                     start=True, stop=True)
                    nc.tensor.matmul(ps_b[:], lhsT=xnT[:],
                                     rhs=w_sb[:, 512:1024],
                                     start=True, stop=True)

                    # Select the block matching each row's label
                    sel = temps.tile([128, 128], F32, tag="sel")
                    nc.vector.tensor_copy(out=sel, in_=ps_a[:, 0:128])
                    # touch ps_b with a 2-wait-capable copy so the 1-wait
                    # CopyPredicated ops below see it already observed
                    pb_probe = statsp.tile([128, 1], F32, tag="pbp")
                    nc.vector.tensor_copy(out=pb_probe, in_=ps_b[:, 0:1])
                    for c in range(1, P):
                        src = ps_a if c < 4 else ps_b
                        blkc = src[:, (c % 4) * 128:(c % 4) * 128 + 128]
                        nc.vector.copy_predicated(
                            out=sel,
                            mask=mask8[:, c, t:t + 1].to_broadcast([128, 128]),
                            data=blkc,
                        )

                    # tanhshrink; tanh lands in the dead x_sb column
                    if t % chw == 0:
                        th = th_scratch[:, t // chw, :]
                    else:
                        th = x_sb[:, t, :]
                    nc.scalar.activation(
                        out=th, in_=sel,
                        func=mybir.ActivationFunctionType.Tanh,
                        bias=zero_t[:, :])
                    o_t = temps.tile([128, 128], F32, tag="o_t")
                    nc.gpsimd.tensor_tensor(out=o_t, in0=sel, in1=th,
                                            op=OP.subtract)
                    nc.sync.dma_start(out=out[ts(t, 128), :], in_=o_t)

    nc.compile()
    return nc


def _get_nc():
    if "nc" not in _NC_CACHE:
        _NC_CACHE["nc"] = _build_kernel()
    return _NC_CACHE["nc"]


def _prep_in_maps(x, W, labels):
    x = np.asarray(x, dtype=np.float32)
    W = np.asarray(W, dtype=np.float32)
    labels = np.asarray(labels)
    w_cat = np.ascontiguousarray(
        W.transpose(0, 2, 1).reshape(C, P * C).astype(np.float32))
    in_maps = []
    for i in range(N_CORES):
        xs = np.ascontiguousarray(x[i * ROWS_PER_CORE:(i + 1) * ROWS_PER_CORE])
        ls = labels[i * ROWS_PER_CORE:(i + 1) * ROWS_PER_CORE]
        lt = np.ascontiguousarray(
            ls.reshape(TILES, 128).T.astype(np.float32))
        in_maps.append({"x": xs, "labels_t": lt, "w_cat": w_cat})
    return in_maps


def run(x, W, labels, trace=False):
    """Run on hardware; returns (output, BassKernelResults)."""
    nc = _get_nc()
    in_maps = _prep_in_maps(x, W, labels)
    res = run_bass_kernel_spmd(nc, in_maps, list(range(N_CORES)), trace=trace)
    outs = [res.results[i]["out"] for i in range(N_CORES)]
    full = np.concatenate(outs, axis=0)
    return full, res


def kernel(x, W, labels):
    full, _ = run(x, W, labels, trace=False)
    return full


# revision 43
# speedup vs baseline: 1.1203x; 1.1203x over previous
"""Trainium2 Bass kernel for nn_CPF_prop_f_87144886436370 (moe_routing).

Per row r of x[N=262144, C=128]:
  xn = (x_r - mean_r) / sqrt(var_r(ddof=1) + 1)
  y  = xn @ W[:, :, labels_r]          (W: [C, C, P=8])
  out_r = y - tanh(y)                   (tanhshrink)

Strategy: data-parallel over 8 NeuronCores (32768 rows each). On each core,
per 128-row tile: layernorm stats + Newton rsqrt + normalize on DVE, PE
transpose, fp32 matmul against all 8 cluster matrices stacked [128, 1024],
per-row selection of the labeled 128-column block via copy_predicated,
tanhshrink (ACT tanh + DVE subtract), store.

Toolchain note: this walrus build allows very few semaphore waits per
instruction, so the kernel is structured to keep every instruction at a
single wait: the x shard is preloaded into SBUF with fresh-region DMAs, PE
warm-up ops absorb one-time cross-engine deps, the ACT engine only ever runs
Tanh (no table switches) and writes into the per-tile dead x_sb column (no
slot rotation → no WAW self-waits), and rsqrt is computed on DVE by Newton
iteration instead of ACT Sqrt.
"""

import numpy as np

import concourse.bass as bass
import concourse.tile as tile
from concourse import bacc, mybir
from concourse.bass import ts
from concourse.bass_utils import run_bass_kernel_spmd
from concourse.masks import make_identity

N = 262144
C = 128
P = 8
N_CORES = 8
ROWS_PER_CORE = N // N_CORES          # 32768
TILES = ROWS_PER_CORE // 128          # 256
FB = 8                                # stats blocking factor
VAR_SCALE = C / (C - 1.0)             # unbiased correction on biased bn var
EPS = 1.0
MAGIC = 0x5F3759DF

F32 = mybir.dt.float32
I32 = mybir.dt.int32
OP = mybir.AluOpType

_NC_CACHE = {}


def _build_kernel():
    # Bacc (not plain Bass): its compile() pass splits semaphore waits to
    # one per instruction, which this walrus build requires.
    nc = bacc.Bacc(target_bir_lowering=False, debug=False)
    x = nc.declare_dram_parameter("x", [ROWS_PER_CORE, C], F32, isOutput=False)
    labels_t = nc.declare_dram_parameter("labels_t", [128, TILES], F32, isOutput=False)
    w_cat = nc.declare_dram_parameter("w_cat", [C, P * C], F32, isOutput=False)
    out = nc.declare_dram_parameter("out", [ROWS_PER_CORE, C], F32, isOutput=True)

    with tile.TileContext(nc) as tc:
        with (
            tc.tile_pool(name="singles", bufs=1) as singles,
            tc.tile_pool(name="temps", bufs=3) as temps,
            tc.tile_pool(name="stats", bufs=3) as statsp,
            tc.tile_pool(name="psum_t", bufs=2, space="PSUM") as psum_t_pool,
            tc.tile_pool(name="psum_mm", bufs=2, space="PSUM") as psum_mm_pool,
            tc.tile_pool(name="psum_w", bufs=1, space="PSUM") as psum_w_pool,
        ):
            # One-time setup
            w_sb = singles.tile([C, P * C], F32)
            nc.sync.dma_start(out=w_sb, in_=w_cat[:, :])
            labels_sb = singles.tile([128, TILES], F32)
            nc.sync.dma_start(out=labels_sb, in_=labels_t[:, :])
            ident = singles.tile([128, 128], F32)
            make_identity(nc, ident[:])
            zero_t = singles.tile([128, 1], F32)
            nc.vector.memset(zero_t[:], 0.0)
            # Per-cluster one-hot masks: mask8[r, c, t] (int mask for
            # CopyPredicated)
            mask8 = singles.tile([128, P, TILES], mybir.dt.uint8)
            for c in range(P):
                nc.vector.tensor_scalar(
                    mask8[:, c, :], labels_sb[:, :], float(c), None,
                    OP.is_equal,
                )

            # Preload the whole x shard into SBUF (64KB/partition) with
            # fresh-region DMAs — no load-slot reuse (DMACopy has a single
            # wait slot and slot-reuse WAW waits would overflow it).
            x_sb = singles.tile([128, TILES, C], F32)
            x_view = x[:, :].rearrange("(t r) c -> r t c", r=128)
            NCH = 16
            chw = TILES // NCH
            for ch in range(NCH):
                nc.sync.dma_start(
                    out=x_sb[:, ch * chw:(ch + 1) * chw, :],
                    in_=x_view[:, ch * chw:(ch + 1) * chw, :])
            # tanh results also go into dead x_sb columns (fresh bytes per
            # tile → no rotating-slot WAW on the ACT engine). Warm the ACT
            # engine on each preload DMA lane so the per-tile tanh carries
            # only its DVE data wait.
            act_warm = singles.tile([128, NCH], F32)
            for ch in range(NCH):
                nc.scalar.copy(out=act_warm[:, ch:ch + 1],
                               in_=x_sb[:, ch * chw, 0:1])
            # tanh for the 16 warm-probed tiles goes to fresh scratch instead
            # (the warm read would otherwise add an ACT WAR wait there)
            th_scratch = singles.tile([128, NCH, 128], F32)

            # PE warm-ups: absorb one-time cross-engine deps (identity from
            # GPSIMD, weights from DMA).
            ps_warm_t = psum_w_pool.tile([128, 128], F32, tag="warm_t")
            nc.tensor.transpose(ps_warm_t[:], ident[:], ident[:])
            ps_warm_m = psum_w_pool.tile([128, 512], F32, tag="warm_m")
            nc.tensor.matmul(ps_warm_m[:], lhsT=w_sb[:, 0:128],
                             rhs=w_sb[:, 0:512], start=True, stop=True)

            n_blocks = TILES // FB
            for blk in range(n_blocks):
                # --- blocked stats: bn stats per tile, rsqrt per block ---
                mv_blk = statsp.tile([128, FB, 2], F32, tag="mv")
                for f in range(FB):
                    t = blk * FB + f
                    stats = statsp.tile([128, 6], F32, tag="bst")
                    nc.vector.bn_stats(out=stats, in_=x_sb[:, t, :])
                    nc.vector.bn_aggr(out=mv_blk[:, f, :], in_=stats)
                # vp = var * C/(C-1) + EPS   [128, FB]
                vp = statsp.tile([128, FB], F32, tag="vp")
                nc.vector.tensor_scalar(
                    vp, mv_blk[:, :, 1], VAR_SCALE, EPS, OP.mult, OP.add)
                # rstd = rsqrt(vp) by magic seed + 3 Newton steps (DVE only)
                vpi = vp[:, :].bitcast(I32)
                yi = statsp.tile([128, FB], I32, tag="yi")
                nc.vector.tensor_scalar(yi, vpi, 1, None, OP.arith_shift_right)
                nc.vector.tensor_scalar(yi, yi, -1, MAGIC, OP.mult, OP.add)
                y = yi[:, :].bitcast(F32)
                tmp = statsp.tile([128, FB], F32, tag="tmp")
                for _ in range(3):
                    nc.vector.tensor_tensor(out=tmp, in0=y, in1=y, op=OP.mult)
                    nc.vector.tensor_tensor(out=tmp, in0=tmp, in1=vp, op=OP.mult)
                    nc.vector.tensor_scalar(tmp, tmp, -0.5, 1.5, OP.mult, OP.add)
                    nc.vector.tensor_tensor(out=y, in0=y, in1=tmp, op=OP.mult)

                for f in range(FB):
                    t = blk * FB + f
                    x_t = x_sb[:, t, :]
                    rstd = y[:, f:f + 1]
                    mean = mv_blk[:, f, 0:1]

                    # xn = (x - mean) * rstd  (GPSIMD — keeps DVE for selection)
                    xn = temps.tile([128, C], F32, tag="xn")
                    nc.gpsimd.tensor_scalar(
                        xn, x_t, mean, rstd, OP.subtract, OP.mult)

                    # Transpose xn -> [C, rows]
                    ps_t = psum_t_pool.tile([128, 128], F32, tag="ps_t")
                    nc.tensor.transpose(ps_t[:], xn[:], ident[:])
                    xnT = temps.tile([128, 128], F32, tag="xnT")
                    nc.scalar.copy(out=xnT, in_=ps_t[:])

                    # Candidates for all 8 clusters: [rows, 8*128]
                    ps_a = psum_mm_pool.tile([128, 512], F32, tag="ps_a")
                    ps_b = psum_mm_pool.tile([128, 512], F32, tag="ps_b")
                    nc.tensor.matmul(ps_a[:], lhsT=xnT[:], rhs=w_sb[:, 0:512],
                                     start=True, stop=True)
                    nc.tensor.matmul(ps_b[:], lhsT=xnT[:],
                                     rhs=w_sb[:, 512:1024],
                                     start=True, stop=True)

                    # Select the block matching each row's label
                    # (Bacc's wait-splitting makes the old ps_b "probe" copy
                    # unnecessary — removed from the per-tile DVE budget.)
                    sel = temps.tile([128, 128], F32, tag="sel")
                    nc.scalar.copy(out=sel, in_=ps_a[:, 0:128])
                    for c in range(1, P):
                        src = ps_a if c < 4 else ps_b
                        blkc = src[:, (c % 4) * 128:(c % 4) * 128 + 128]
                        nc.vector.copy_predicated(
                            out=sel,
                            mask=mask8[:, c, t:t + 1].to_broadcast([128, 128]),
                            data=blkc,
                        )

                    # tanhshrink; tanh lands in the dead x_sb column
                    if t % chw == 0:
                        th = th_scratch[:, t // chw, :]
                    else:
                        th = x_sb[:, t, :]
                    nc.scalar.activation(
                        out=th, in_=sel,
                        func=mybir.ActivationFunctionType.Tanh,
                        bias=zero_t[:, :])
                    o_t = temps.tile([128, 128], F32, tag="o_t")
                    nc.gpsimd.tensor_tensor(out=o_t, in0=sel, in1=th,
                                            op=OP.subtract)
                    nc.sync.dma_start(out=out[ts(t, 128), :], in_=o_t)

    nc.compile()
    return nc


def _get_nc():
    if "nc" not in _NC_CACHE:
        _NC_CACHE["nc"] = _build_kernel()
    return _NC_CACHE["nc"]


def _prep_in_maps(x, W, labels):
    x = np.asarray(x, dtype=np.float32)
    W = np.asarray(W, dtype=np.float32)
    labels = np.asarray(labels)
    w_cat = np.ascontiguousarray(
        W.transpose(0, 2, 1).reshape(C, P * C).astype(np.float32))
    in_maps = []
    for i in range(N_CORES):
        xs = np.ascontiguousarray(x[i * ROWS_PER_CORE:(i + 1) * ROWS_PER_CORE])
        ls = labels[i * ROWS_PER_CORE:(i + 1) * ROWS_PER_CORE]
        lt = np.ascontiguousarray(
            ls.reshape(TILES, 128).T.astype(np.float32))
        in_maps.append({"x": xs, "labels_t": lt, "w_cat": w_cat})
    return in_maps


def run(x, W, labels, trace=False):
    """Run on hardware; returns (output, BassKernelResults)."""
    nc = _get_nc()
    in_maps = _prep_in_maps(x, W, labels)
    res = run_bass_kernel_spmd(nc, in_maps, list(range(N_CORES)), trace=trace)
    outs = [res.results[i]["out"] for i in range(N_CORES)]
    full = np.concatenate(outs, axis=0)
    return full, res


def kernel(x, W, labels):
    full, _ = run(x, W, labels, trace=False)
    return full
